# revision 14
# baseline (speedup 1.0000x reference)
from contextlib import ExitStack

import numpy as np
import ml_dtypes

import concourse.bass as bass
import concourse.tile as tile
from concourse import bacc, mybir
from concourse import bass_utils
from concourse.masks import make_identity

BF16 = ml_dtypes.bfloat16
NCORES = 8
NUM_ENT = 100000
NUM_REL = 400
D_IN = 200
D_OUT = 400
E = 600000
HALF = E // 2
B = 1024
P = 128
SHARD = 12500          # real entities per core
SHARDP = 12544         # padded to 98*128
AGG_ROWS = SHARDP * 8  # 100352
TRASH = 12500          # trash agg row (core 0 pad region)
VS = 500               # decoder column slice
NV = 12500             # entities per core for decoder
BN_EPS = 1e-5
OUT_SCALE = 254.0
LAST_RUN_S = None

F32 = mybir.dt.float32
I32 = mybir.dt.int32
BF = mybir.dt.bfloat16
U8 = mybir.dt.uint8
F8 = mybir.dt.float8e4
EW_SCALE = 8.0


def _prow(v):
    # entity id -> padded row in the allgathered [100352, 200] layout
    return (v // SHARD) * SHARDP + (v % SHARD)


def _pad2(w):
    # [200, 400] -> [2, 128, 400] zero padded on k
    out = np.zeros((2, P, D_OUT), np.float32)
    out[0] = w[:P]
    out[1, : D_IN - P] = w[P:]
    return out


def _prep(inputs):
    src = np.asarray(inputs["src"]).astype(np.int64)
    dst = np.asarray(inputs["dst"]).astype(np.int64)
    et = np.asarray(inputs["edge_type"]).astype(np.int64)
    norm = np.asarray(inputs["edge_norm"]).astype(np.float32)
    rel_emb = np.asarray(inputs["rel_emb"]).astype(np.float32)
    loop_rel = np.asarray(inputs["loop_rel"]).astype(np.float32)[0]

    dirs = (np.arange(E) >= HALF).astype(np.int64)

    # group edges by (dir, rel) via one argsort
    key = dirs * NUM_REL + et
    order = np.argsort(key, kind="stable")
    skey = key[order]
    bounds = np.flatnonzero(np.diff(skey)) + 1
    starts = np.r_[0, bounds]
    ends = np.r_[bounds, E]
    gkeys = skey[starts]

    # greedy assign groups to cores per dir (desc by size)
    per_core_groups = [[[], []] for _ in range(NCORES)]  # [core][dir] -> (eids, rel)
    for d in range(2):
        gsel = np.flatnonzero((gkeys >= d * NUM_REL) & (gkeys < (d + 1) * NUM_REL))
        sizes = ends[gsel] - starts[gsel]
        o = np.argsort(-sizes, kind="stable")
        loads = [0] * NCORES
        for gi in gsel[o]:
            c = int(np.argmin(loads))
            n = int(ends[gi] - starts[gi])
            loads[c] += n
            per_core_groups[c][d].append(
                (order[starts[gi]:ends[gi]], int(gkeys[gi] - d * NUM_REL)))
    for c in range(NCORES):
        for d in range(2):
            per_core_groups[c][d].sort(key=lambda t: -len(t[0]))

    NS = [max(len(per_core_groups[c][d]) for c in range(NCORES)) for d in range(2)]
    TPG = []
    for d in range(2):
        tp = []
        for i in range(NS[d]):
            mx = 0
            for c in range(NCORES):
                gl = per_core_groups[c][d]
                if i < len(gl):
                    mx = max(mx, (len(gl[i][0]) + P - 1) // P)
            tp.append(mx)
        TPG.append(tp)
    T1 = sum(TPG[0]) + sum(TPG[1])

    # phase-2 packing per core (vectorized): sort core edges by dst,
    # pack dst-runs into 128-slot tiles
    packs = []
    T2s = []
    for c in range(NCORES):
        glist = [g for d in range(2) for (g, r) in per_core_groups[c][d]]
        eids = np.concatenate(glist) if glist else np.zeros(0, np.int64)
        o2 = np.argsort(dst[eids], kind="stable")
        e2 = eids[o2]
        dv = dst[e2]
        nE = len(e2)
        chg = np.flatnonzero(np.diff(dv)) + 1
        rst = np.r_[0, chg]
        ren = np.r_[chg, nE]
        sizes = ren - rst
        nruns = len(sizes)
        cum = np.cumsum(sizes)
        # sequential first-fit packing of runs
        lo_list = []
        pos = 0
        prev = 0
        while pos < nruns:
            hi = int(np.searchsorted(cum, prev + P, side="right"))
            assert hi > pos, "dst run larger than 128"
            lo_list.append(pos)
            prev = int(cum[hi - 1])
            pos = hi
        T2 = len(lo_list)
        lo_arr = np.asarray(lo_list, np.int64)
        runs_per_tile = np.r_[lo_arr[1:], nruns] - lo_arr
        tile_of_run = np.repeat(np.arange(T2), runs_per_tile)
        rank_of_run = np.arange(nruns) - lo_arr[tile_of_run]
        run_of_edge = np.repeat(np.arange(nruns), sizes)
        tile_of_edge = tile_of_run[run_of_edge]
        edge_start_of_tile = rst[lo_arr]
        pos_of_edge = np.arange(nE) - edge_start_of_tile[tile_of_edge]
        mpos_e2 = tile_of_edge * P + pos_of_edge
        seg_e2 = rank_of_run[run_of_edge]
        packs.append((e2, mpos_e2, seg_e2, tile_of_run, rank_of_run,
                      dv[rst]))
        T2s.append(T2)
    T = max(T1, max(T2s))
    T2MAX = max(T2s)

    data = []
    for c in range(NCORES):
        e2, mpos_e2, seg_e2, tile_of_run, rank_of_run, run_dst = packs[c]
        seg = np.full(T * P, 127, np.float32)
        seg[mpos_e2] = seg_e2
        vout = np.full(T * P, TRASH, np.int32)
        vout[tile_of_run * P + rank_of_run] = _prow(run_dst)
        mpos_by_eid = np.zeros(E, np.int32)
        mpos_by_eid[e2] = mpos_e2
        used = np.zeros(T * P, bool)
        used[mpos_e2] = True
        pad_slots = np.flatnonzero(~used).astype(np.int32)

        srcA = np.zeros(T * P, np.int32)
        nrmA = np.zeros(T * P, np.float32)
        mposA = np.zeros(T * P, np.int32)
        relS = np.zeros((NS[0] + NS[1] + 2, D_OUT), np.float32)
        q = 0
        pp = 0  # pad_slots cursor
        for d in range(2):
            gl = per_core_groups[c][d]
            base = 0 if d == 0 else NS[0]
            for i in range(NS[d]):
                if i < len(gl):
                    g, r = gl[i]
                    relS[base + i, :D_IN] = rel_emb[r]
                    relS[base + i, D_IN:] = rel_emb[r]
                    nreal = len(g)
                else:
                    nreal = 0
                ntile = TPG[d][i]
                if nreal:
                    srcA[q:q + nreal] = _prow(src[g])
                    nrmA[q:q + nreal] = norm[g] / 3.0
                    mposA[q:q + nreal] = mpos_by_eid[g]
                npad = ntile * P - nreal
                if npad:
                    mposA[q + nreal:q + ntile * P] = pad_slots[pp:pp + npad]
                    pp += npad
                q += ntile * P
        nrem = T * P - q
        if nrem:
            mposA[q:] = pad_slots[pp:pp + nrem]
            pp += nrem
        assert pp == len(pad_slots), f"pad mismatch {pp} {len(pad_slots)}"
        relS[NS[0] + NS[1], :D_IN] = loop_rel
        relS[NS[0] + NS[1], D_IN:] = loop_rel

        vmask = np.zeros((SHARDP, 1), np.float32)
        vmask[:SHARD] = 1.0
        triples = np.asarray(inputs["triples"]).astype(np.int64)
        head = triples[:, 0]
        hidx = np.zeros((B, 1), np.int32)
        hmask = np.zeros((B, 1), np.float32)
        own = head // SHARD == c
        hidx[own, 0] = (head[own] % SHARD).astype(np.int32)
        hmask[own, 0] = 1.0

        def lay(a, dt):
            return np.ascontiguousarray(a.reshape(T, P).T.astype(dt))

        data.append(dict(
            srcA=lay(srcA, np.int32), nrmA=lay(nrmA, np.float32),
            mposA=lay(mposA, np.int32), segA=lay(seg, np.float32),
            voutA=lay(vout, np.int32), relS=relS.astype(BF16),
            vmask=vmask, hidx=hidx, hmask=hmask))
    return data, NS, TPG, T, T2MAX


def _build(NS, TPG, T, T2MAX=None):
    nc = bacc.Bacc("TRN2", target_bir_lowering=False, debug=False,
                   num_devices=NCORES)
    NSLOT = NS[0] + NS[1] + 1
    entp = nc.dram_tensor("entp", [SHARDP, D_IN], F8, kind="ExternalInput")
    relS = nc.dram_tensor("relS", [NSLOT + 1, D_OUT], BF, kind="ExternalInput")
    w_in = nc.dram_tensor("w_in", [2, P, D_OUT], BF, kind="ExternalInput")
    w_out = nc.dram_tensor("w_out", [2, P, D_OUT], BF, kind="ExternalInput")
    w_loop = nc.dram_tensor("w_loop", [2, P, D_OUT], BF, kind="ExternalInput")
    relT = nc.dram_tensor("relT", [2, P, NUM_REL], BF, kind="ExternalInput")
    wrel = nc.dram_tensor("wrel", [2, P, D_OUT], BF, kind="ExternalInput")
    srcA = nc.dram_tensor("srcA", [P, T], I32, kind="ExternalInput")
    nrmA = nc.dram_tensor("nrmA", [P, T], F32, kind="ExternalInput")
    mposA = nc.dram_tensor("mposA", [P, T], I32, kind="ExternalInput")
    segA = nc.dram_tensor("segA", [P, T], F32, kind="ExternalInput")
    voutA = nc.dram_tensor("voutA", [P, T], I32, kind="ExternalInput")
    vmaskA = nc.dram_tensor("vmaskA", [SHARDP, 1], F32, kind="ExternalInput")
    hidxA = nc.dram_tensor("hidxA", [B, 1], I32, kind="ExternalInput")
    hmaskA = nc.dram_tensor("hmaskA", [B, 1], F32, kind="ExternalInput")
    relaA = nc.dram_tensor("relaA", [B, 1], I32, kind="ExternalInput")
    gamma = nc.dram_tensor("gamma", [1, D_OUT], F32, kind="ExternalInput")
    beta = nc.dram_tensor("beta", [1, D_OUT], F32, kind="ExternalInput")
    embw = nc.dram_tensor("embw", [P, 4 * NV], F8, kind="ExternalInput")
    score = nc.dram_tensor("score", [B, NV], U8, kind="ExternalOutput")

    with tile.TileContext(nc) as tc, ExitStack() as ctx:
        sb = ctx.enter_context(tc.tile_pool(name="sb", bufs=4))
        cst = ctx.enter_context(tc.tile_pool(name="cst", bufs=1))
        dec = ctx.enter_context(tc.tile_pool(name="dec", bufs=2))
        ewp = ctx.enter_context(tc.tile_pool(name="ewp", bufs=4))
        pp = ctx.enter_context(tc.tile_pool(name="pp", bufs=3, space="PSUM"))
        ppb = ctx.enter_context(tc.tile_pool(name="ppb", bufs=1, space="PSUM"))
        pst = ctx.enter_context(tc.tile_pool(name="pst", bufs=1, space="PSUM"))
        dram = ctx.enter_context(tc.tile_pool(name="dram", bufs=1, space="DRAM"))

        entl = dram.tile([SHARDP, D_IN], F8, tag="entl")
        ag = dram.tile([AGG_ROWS, D_IN], F8, tag="ag")
        msg_d = dram.tile([T * P, D_OUT], BF, tag="msg_d")
        pagg = dram.tile([AGG_ROWS, D_OUT], BF, tag="pagg")
        ragg = dram.tile([SHARDP, D_OUT], BF, tag="ragg")
        x_d = dram.tile([SHARDP, D_OUT], BF, tag="x_d")
        hx_l = dram.tile([B, D_OUT], BF, tag="hx_l")
        hx_f = dram.tile([B, D_OUT], BF, tag="hx_f")
        r_d = dram.tile([NUM_REL, D_OUT], BF, tag="r_d")
        st_l = dram.tile([1, 2 * D_OUT], F32, tag="st_l")
        st_f = dram.tile([1, 2 * D_OUT], F32, tag="st_f")

        identf = cst.tile([P, P], F32, tag="identf")
        make_identity(nc, identf[:])
        identb = cst.tile([P, P], BF, tag="identb")
        make_identity(nc, identb[:])
        ident8 = cst.tile([P, P], F8, tag="ident8")
        make_identity(nc, ident8[:])
        iota_i = cst.tile([P, P], I32, tag="iota_i")
        nc.gpsimd.iota(iota_i[:], [[1, P]], base=0, channel_multiplier=0)
        iota_f = cst.tile([P, P], F32, tag="iota_f")
        nc.vector.tensor_copy(iota_f[:], iota_i[:])
        ones_r = cst.tile([1, P], BF, tag="ones_r")
        nc.gpsimd.memset(ones_r[:], 1.0)
        zero_sb = cst.tile([P, 3200], BF, tag="zero_sb")
        nc.gpsimd.memset(zero_sb[:], 0.0)

        # batched per-tile metadata, resident in SBUF
        srcS = cst.tile([P, T], I32, tag="srcS")
        nc.sync.dma_start(srcS[:], srcA[:, :])
        nrmS = cst.tile([P, T], F32, tag="nrmS")
        nc.sync.dma_start(nrmS[:], nrmA[:, :])
        mposS = cst.tile([P, T], I32, tag="mposS")
        nc.sync.dma_start(mposS[:], mposA[:, :])
        segS = cst.tile([P, T], F32, tag="segS")
        nc.sync.dma_start(segS[:], segA[:, :])
        voutS = cst.tile([P, T], I32, tag="voutS")
        nc.sync.dma_start(voutS[:], voutA[:, :])

        # local ent shard -> internal, then allgather full ent (bf16)
        nc.sync.dma_start(entl[:, :], entp[:, :])
        nc.gpsimd.collective_compute(
            "AllGather", mybir.AluOpType.bypass,
            replica_groups=[list(range(NCORES))],
            ins=[entl.opt()], outs=[ag.opt()])

        # zero partial agg (100352*400 bf16 = 80MB)
        rows_per = 1024  # [128, 3200] covers 1024 rows of 400
        for i in range(AGG_ROWS // rows_per):
            nc.sync.dma_start(
                bass.AP(pagg.tensor, i * rows_per * D_OUT, [[3200, P], [1, 3200]]),
                zero_sb[:])

        # W tiles resident
        def load_w(t):
            w = cst.tile([P, 2 * D_OUT], BF, tag=f"w{t.name}")
            nc.sync.dma_start(w[:, 0:D_OUT], t[0, :, :])
            nc.sync.dma_start(w[:, D_OUT:2 * D_OUT], t[1, :, :])
            return w
        w_in_sb = load_w(w_in)
        w_out_sb = load_w(w_out)
        w_loop_sb = load_w(w_loop)

        relS_t = relS[0:1, 0:1].tensor

        def build_mt(slot, w_sb, scale=1.0):
            # circulant rows via strided DMA: ct[t, kc*200+j] = relS[slot, kc*128+t+j]
            ct = sb.tile([P, 2 * D_IN], BF, tag="ct")
            nc.sync.dma_start(
                ct[:, 0:D_IN],
                bass.AP(relS_t, slot * D_OUT, [[1, P], [1, D_IN]]))
            nc.sync.dma_start(
                ct[:, D_IN:2 * D_IN],
                bass.AP(relS_t, slot * D_OUT + P, [[1, P], [1, D_IN]]))
            mt = sb.tile([P, 2 * D_OUT], BF, tag="mt")
            for jc in range(2):
                js = P if jc == 0 else D_IN - P
                mps = pp.tile([P, D_OUT], F32, tag="mm", space="PSUM")
                for kc in range(2):
                    nc.tensor.matmul(
                        out=mps[:js, :],
                        lhsT=ct[:, kc * D_IN + jc * P:kc * D_IN + jc * P + js],
                        rhs=w_sb[:, kc * D_OUT:(kc + 1) * D_OUT],
                        start=(kc == 0), stop=(kc == 1))
                nc.scalar.activation(mt[:js, jc * D_OUT:(jc + 1) * D_OUT],
                                     mps[:js, :],
                                     mybir.ActivationFunctionType.Copy, scale=scale)
            return mt

        # ---------------- phase 1: messages ----------------
        def p1_tile(q, mt):
            a = sb.tile([P, D_IN], F8, tag="a")
            nc.gpsimd.indirect_dma_start(
                out=a[:], out_offset=None, in_=ag[:, :],
                in_offset=bass.IndirectOffsetOnAxis(ap=srcS[:, q:q + 1], axis=0))
            at = sb.tile([P, 2 * P], BF, tag="at")
            for jc in range(2):
                js = P if jc == 0 else D_IN - P
                tp = ppb.tile([P, 2 * P], F8, tag="mmb8", space="PSUM")
                nc.tensor.transpose(out=tp[:js, ::2], in_=a[:, jc * P:jc * P + js],
                                    identity=ident8[:])
                nc.vector.tensor_copy(at[:js, jc * P:(jc + 1) * P], tp[:js, ::2])
            mps = pp.tile([P, D_OUT], F32, tag="mm", space="PSUM")
            for jc in range(2):
                js = P if jc == 0 else D_IN - P
                nc.tensor.matmul(out=mps[:], lhsT=at[:js, jc * P:(jc + 1) * P],
                                 rhs=mt[:js, jc * D_OUT:(jc + 1) * D_OUT],
                                 start=(jc == 0), stop=(jc == 1))
            mb = sb.tile([P, D_OUT], BF, tag="mb")
            nc.scalar.activation(mb[:], mps[:],
                                 mybir.ActivationFunctionType.Copy,
                                 scale=nrmS[:, q:q + 1])
            nc.gpsimd.indirect_dma_start(
                out=msg_d[:, :],
                out_offset=bass.IndirectOffsetOnAxis(ap=mposS[:, q:q + 1], axis=0),
                in_=mb[:], in_offset=None)

        q = 0
        mt = None
        for d in range(2):
            w_sb = w_in_sb if d == 0 else w_out_sb
            base = 0 if d == 0 else NS[0]
            for i in range(NS[d]):
                if TPG[d][i] == 0:
                    continue
                mt = build_mt(base + i, w_sb)
                for _ in range(TPG[d][i]):
                    p1_tile(q, mt)
                    q += 1
        while q < T:
            p1_tile(q, mt)
            q += 1

        # ---------------- phase 2: segment sum ----------------
        # tiles beyond T2MAX only sum zero pad messages into the trash row
        for t in range(T2MAX if T2MAX is not None else T):
            mrows = sb.tile([P, D_OUT], BF, tag="mrows")
            nc.sync.dma_start(mrows[:], msg_d[t * P:(t + 1) * P, :])
            S = sb.tile([P, P], BF, tag="S")
            nc.vector.tensor_scalar(S[:], iota_f[:], segS[:, t:t + 1], None,
                                    op0=mybir.AluOpType.is_equal)
            ps = pp.tile([P, D_OUT], F32, tag="mm", space="PSUM")
            nc.tensor.matmul(out=ps[:], lhsT=S[:], rhs=mrows[:],
                             start=True, stop=True)
            ab = sb.tile([P, D_OUT], BF, tag="ab")
            nc.scalar.activation(ab[:], ps[:],
                                 mybir.ActivationFunctionType.Copy)
            nc.gpsimd.indirect_dma_start(
                out=pagg[:, :],
                out_offset=bass.IndirectOffsetOnAxis(ap=voutS[:, t:t + 1], axis=0),
                in_=ab[:], in_offset=None)

        # reduce-scatter partial agg -> local shard
        nc.gpsimd.collective_compute(
            "ReduceScatter", mybir.AluOpType.add,
            replica_groups=[list(range(NCORES))],
            ins=[pagg.opt()], outs=[ragg.opt()])

        # ---------------- x = agg + loop, stats ----------------
        ml = build_mt(NS[0] + NS[1], w_loop_sb, scale=1.0 / 3.0)
        ps1 = pst.tile([1, D_OUT], F32, tag="ps1", space="PSUM")
        ps2 = pst.tile([1, D_OUT], F32, tag="ps2", space="PSUM")
        NT = SHARDP // P
        for t in range(NT):
            vm = sb.tile([P, 1], F32, tag="vm")
            nc.sync.dma_start(vm[:], vmaskA[t * P:(t + 1) * P, :])
            vmb = sb.tile([P, 1], BF, tag="vmb")
            nc.vector.tensor_copy(vmb[:], vm[:])
            av = sb.tile([P, D_IN], F8, tag="a")
            nc.sync.dma_start(av[:], entp[t * P:(t + 1) * P, :])
            at = sb.tile([P, 2 * P], BF, tag="at")
            for jc in range(2):
                js = P if jc == 0 else D_IN - P
                tp = ppb.tile([P, 2 * P], F8, tag="mmb8", space="PSUM")
                nc.tensor.transpose(out=tp[:js, ::2], in_=av[:, jc * P:jc * P + js],
                                    identity=ident8[:])
                nc.vector.tensor_copy(at[:js, jc * P:(jc + 1) * P], tp[:js, ::2])
            lp = pp.tile([P, D_OUT], F32, tag="mm", space="PSUM")
            for jc in range(2):
                js = P if jc == 0 else D_IN - P
                nc.tensor.matmul(out=lp[:], lhsT=at[:js, jc * P:(jc + 1) * P],
                                 rhs=ml[:js, jc * D_OUT:(jc + 1) * D_OUT],
                                 start=(jc == 0), stop=(jc == 1))
            ag2 = sb.tile([P, D_OUT], BF, tag="ag2")
            nc.sync.dma_start(ag2[:], ragg[t * P:(t + 1) * P, :])
            xb = sb.tile([P, D_OUT], BF, tag="xb")
            nc.vector.tensor_add(xb[:], ag2[:], lp[:])
            nc.sync.dma_start(x_d[t * P:(t + 1) * P, :], xb[:])
            xs = sb.tile([P, D_OUT], BF, tag="xs")
            nc.vector.tensor_mul(xs[:], xb[:], xb[:])
            nc.tensor.matmul(out=ps1[:], lhsT=vmb[:], rhs=xb[:],
                             start=(t == 0), stop=(t == NT - 1))
            nc.tensor.matmul(out=ps2[:], lhsT=vmb[:], rhs=xs[:],
                             start=(t == 0), stop=(t == NT - 1))
        stl = sb.tile([1, 2 * D_OUT], F32, tag="stl")
        nc.vector.tensor_copy(stl[:, 0:D_OUT], ps1[:])
        nc.vector.tensor_copy(stl[:, D_OUT:2 * D_OUT], ps2[:])
        nc.sync.dma_start(st_l[:, :], stl[:])
        nc.gpsimd.collective_compute(
            "AllReduce", mybir.AluOpType.add,
            replica_groups=[list(range(NCORES))],
            ins=[st_l.opt()], outs=[st_f.opt()])

        # s = gamma / sqrt(var+eps), b = beta - mean*s
        stf = sb.tile([1, 2 * D_OUT], F32, tag="stf")
        nc.sync.dma_start(stf[:], st_f[:, :])
        mean = sb.tile([1, D_OUT], F32, tag="mean")
        nc.vector.tensor_scalar_mul(mean[:], stf[:, 0:D_OUT], 1.0 / NUM_ENT)
        var = sb.tile([1, D_OUT], F32, tag="var")
        nc.vector.tensor_scalar_mul(var[:], stf[:, D_OUT:2 * D_OUT], 1.0 / NUM_ENT)
        m2 = sb.tile([1, D_OUT], F32, tag="m2")
        nc.vector.tensor_mul(m2[:], mean[:], mean[:])
        nc.vector.tensor_sub(var[:], var[:], m2[:])
        nc.vector.tensor_scalar_add(var[:], var[:], BN_EPS)
        sd = sb.tile([1, D_OUT], F32, tag="sd")
        nc.scalar.sqrt(sd[:], var[:])
        rsd = sb.tile([1, D_OUT], F32, tag="rsd")
        nc.vector.reciprocal(rsd[:], sd[:])
        gm = sb.tile([1, D_OUT], F32, tag="gm")
        nc.sync.dma_start(gm[:], gamma[:, :])
        bt = sb.tile([1, D_OUT], F32, tag="bt")
        nc.sync.dma_start(bt[:], beta[:, :])
        sv = sb.tile([1, D_OUT], BF, tag="sv")
        nc.vector.tensor_mul(sv[:], gm[:], rsd[:])
        svf = sb.tile([1, D_OUT], F32, tag="svf")
        nc.vector.tensor_copy(svf[:], sv[:])
        bv = sb.tile([1, D_OUT], BF, tag="bv")
        ms = sb.tile([1, D_OUT], F32, tag="ms")
        nc.vector.tensor_mul(ms[:], mean[:], svf[:])
        nc.vector.tensor_sub(bv[:], bt[:], ms[:])
        # broadcast to [128, 400]
        sR = sb.tile([P, D_OUT], BF, tag="sR")
        bR = sb.tile([P, D_OUT], BF, tag="bR")
        for srcv, dstv in ((sv, sR), (bv, bR)):
            pb = pp.tile([P, D_OUT], F32, tag="mm", space="PSUM")
            nc.tensor.matmul(out=pb[:], lhsT=ones_r[:1, :], rhs=srcv[:1, :],
                             start=True, stop=True)
            nc.vector.tensor_copy(dstv[:], pb[:])

        # r = rel_emb @ w_rel -> r_d
        wr = load_w(wrel)
        rT = cst.tile([P, 2 * NUM_REL], BF, tag="rT")
        nc.sync.dma_start(rT[:, 0:NUM_REL], relT[0, :, :])
        nc.sync.dma_start(rT[:, NUM_REL:2 * NUM_REL], relT[1, :, :])
        for mc in range(4):
            pr = pp.tile([P, D_OUT], F32, tag="mm", space="PSUM")
            for kc in range(2):
                nc.tensor.matmul(
                    out=pr[:100, :],
                    lhsT=rT[:, kc * NUM_REL + mc * 100:kc * NUM_REL + (mc + 1) * 100],
                    rhs=wr[:, kc * D_OUT:(kc + 1) * D_OUT],
                    start=(kc == 0), stop=(kc == 1))
            rb_ = sb.tile([P, D_OUT], BF, tag="rb_")
            nc.scalar.activation(rb_[:100, :], pr[:100, :],
                                 mybir.ActivationFunctionType.Copy)
            nc.sync.dma_start(r_d[mc * 100:(mc + 1) * 100, :], rb_[:100, :])

        # heads: gather x rows, BN+tanh, mask, assemble
        for t in range(B // P):
            hi = sb.tile([P, 1], I32, tag="hi")
            nc.sync.dma_start(hi[:], hidxA[t * P:(t + 1) * P, :])
            hm = sb.tile([P, 1], F32, tag="hm")
            nc.sync.dma_start(hm[:], hmaskA[t * P:(t + 1) * P, :])
            xg = sb.tile([P, D_OUT], BF, tag="xg")
            nc.gpsimd.indirect_dma_start(
                out=xg[:], out_offset=None, in_=x_d[:, :],
                in_offset=bass.IndirectOffsetOnAxis(ap=hi[:, :1], axis=0))
            xn = sb.tile([P, D_OUT], BF, tag="xn")
            nc.vector.tensor_mul(xn[:], xg[:], sR[:])
            nc.vector.tensor_add(xn[:], xn[:], bR[:])
            xt = sb.tile([P, D_OUT], BF, tag="xt")
            nc.scalar.activation(xt[:], xn[:], mybir.ActivationFunctionType.Tanh)
            hx = sb.tile([P, D_OUT], BF, tag="hx")
            nc.vector.tensor_scalar_mul(hx[:], xt[:], hm[:, :1])
            nc.sync.dma_start(hx_l[t * P:(t + 1) * P, :], hx[:])
        nc.gpsimd.collective_compute(
            "AllReduce", mybir.AluOpType.add,
            replica_groups=[list(range(NCORES))],
            ins=[hx_l.opt()], outs=[hx_f.opt()])

        # obj = hx * r[rela]; objT chunks
        objT = []
        for t in range(B // P):
            ra = sb.tile([P, 1], I32, tag="ra")
            nc.sync.dma_start(ra[:], relaA[t * P:(t + 1) * P, :])
            rr = sb.tile([P, D_OUT], BF, tag="rr")
            nc.gpsimd.indirect_dma_start(
                out=rr[:], out_offset=None, in_=r_d[:, :],
                in_offset=bass.IndirectOffsetOnAxis(ap=ra[:, :1], axis=0))
            hh = sb.tile([P, D_OUT], BF, tag="hh")
            nc.sync.dma_start(hh[:], hx_f[t * P:(t + 1) * P, :])
            ob = sb.tile([P, D_OUT], BF, tag="ob")
            nc.vector.tensor_mul(ob[:], hh[:], rr[:])
            row = []
            for fc in range(4):
                tp = ppb.tile([P, P], BF, tag="mmb", space="PSUM")
                nc.tensor.transpose(out=tp[:100, :],
                                    in_=ob[:, fc * 100:(fc + 1) * 100],
                                    identity=identb[:])
                ot = cst.tile([101, P], BF, tag=f"ot{t}_{fc}")
                if fc == 0:
                    nc.gpsimd.memset(ot[:], 1.0)
                nc.vector.tensor_copy(ot[:100, :], tp[:100, :])
                row.append(ot)
            objT.append(row)

        # decoder -> uint8 score (embw fp8, prescaled by 8; bias in fc0 row 100)
        embw_t = embw[0:1, 0:1].tensor
        for v in range(NV // VS):
            ew4 = ewp.tile([P, 4 * VS], F8, tag="ew")
            nc.sync.dma_start(
                ew4[:],
                bass.AP(embw_t, v * VS, [[4 * NV, P], [NV, 4], [1, VS]]))
            for t in range(B // P):
                pd = pp.tile([P, VS], F32, tag="mm", space="PSUM")
                nc.tensor.matmul(out=pd[:], lhsT=objT[t][0][:, :],
                                 rhs=ew4[:101, 0:VS], start=True, stop=False)
                for fc in range(1, 4):
                    nc.tensor.matmul(out=pd[:], lhsT=objT[t][fc][:100, :],
                                     rhs=ew4[:100, fc * VS:(fc + 1) * VS],
                                     start=False, stop=(fc == 3))
                sc = dec.tile([P, VS], F32, tag="sc")
                nc.scalar.activation(sc[:], pd[:],
                                     mybir.ActivationFunctionType.Sigmoid,
                                     scale=1.0 / EW_SCALE)
                su = dec.tile([P, VS], U8, tag="su")
                nc.vector.tensor_scalar(su[:], sc[:], OUT_SCALE, 0.5,
                                        op0=mybir.AluOpType.mult,
                                        op1=mybir.AluOpType.add)
                nc.sync.dma_start(score[t * P:(t + 1) * P, v * VS:(v + 1) * VS],
                                  su[:])
    nc.compile()
    return nc


def kernel(**inputs):
    data, NS, TPG, T, T2MAX = _prep(inputs)
    nc = _build(NS, TPG, T, T2MAX)

    ent = np.asarray(inputs["ent_emb"], np.float32)
    entp_all = np.zeros((NCORES, SHARDP, D_IN), np.float32)
    for c in range(NCORES):
        entp_all[c, :SHARD] = ent[c * SHARD:(c + 1) * SHARD]
    entp_all = entp_all.astype(mybir.dt.np(F8))
    w_in = _pad2(np.asarray(inputs["in_w"], np.float32)).astype(BF16)
    w_out = _pad2(np.asarray(inputs["out_w"], np.float32)).astype(BF16)
    w_loop = _pad2(np.asarray(inputs["loop_w"], np.float32)).astype(BF16)
    wrel = _pad2(np.asarray(inputs["w_rel"], np.float32)).astype(BF16)
    relT = np.zeros((2, P, NUM_REL), np.float32)
    re = np.asarray(inputs["rel_emb"], np.float32).T  # [200, 400]
    relT[0] = re[:P]
    relT[1, : D_IN - P] = re[P:]
    relT = relT.astype(BF16)
    gamma = np.asarray(inputs["bn_gamma"], np.float32).reshape(1, D_OUT)
    beta = np.asarray(inputs["bn_beta"], np.float32).reshape(1, D_OUT)
    rela = np.asarray(inputs["triples"])[:, 1].astype(np.int32).reshape(B, 1)
    ew_full = np.asarray(inputs["emb_ent_w"], np.float32)  # [100000, 400]
    ebias_full = np.asarray(inputs["ent_bias"], np.float32)

    in_maps = []
    for c in range(NCORES):
        d = data[c]
        sl = slice(c * NV, (c + 1) * NV)
        embw = np.zeros((4, P, NV), np.float32)
        ewT = ew_full[sl].T  # [400, 12500]
        for fc in range(4):
            embw[fc, :100] = ewT[fc * 100:(fc + 1) * 100] * EW_SCALE
        embw[0, 100] = ebias_full[sl] * EW_SCALE
        embw = np.concatenate([embw[fc] for fc in range(4)], axis=1)
        in_maps.append({
            "entp": entp_all[c], "relS": d["relS"], "w_in": w_in,
            "w_out": w_out, "w_loop": w_loop, "relT": relT, "wrel": wrel,
            "srcA": d["srcA"], "nrmA": d["nrmA"], "mposA": d["mposA"],
            "segA": d["segA"], "voutA": d["voutA"],
            "vmaskA": d["vmask"], "hidxA": d["hidx"], "hmaskA": d["hmask"],
            "relaA": rela, "gamma": gamma, "beta": beta,
            "embw": embw.astype(mybir.dt.np(F8)),
        })

    import time as _time
    _t0 = _time.time()
    res = bass_utils.run_bass_kernel_spmd(nc, in_maps,
                                          core_ids=list(range(NCORES)))
    global LAST_RUN_S
    LAST_RUN_S = _time.time() - _t0
    out = np.concatenate([res.results[c]["score"] for c in range(NCORES)],
                         axis=1)
    return out.astype(np.float32) * np.float32(1.0 / OUT_SCALE)
